# revision 1
# baseline (speedup 1.0000x reference)
"""NodeAttention (GNN scatter-softmax attention) on 8 Trainium2 NeuronCores.

Strategy:
- Host deals nodes to 8 cores round-robin by degree rank, so every core sees a
  near-identical degree profile; one static NEFF serves all cores (SPMD).
- Per core: 49 node-tiles x 128 nodes; node-tile t gets a dense padded slot
  grid [128, D_t] (D_t = max degree in tile across cores; ~3% padding).
- No gather at all: the host replicates x per SLOT (xTc column per edge slot,
  inverse-permuted so the device's KV build lands row r at slot order
  r = tile_base + p*D + k). The device builds the slot-ordered bf16 KV=(K|V)
  table in DRAM via matmuls, and each node-tile reads its KV rows back with a
  plain full-rate DMA. Build and edge phases are interleaved per 4096-row
  batch so compute starts ~immediately.
- Per-edge scores: bf16 QK muls on DVE, per-edge bias via block-diagonal
  matmuls (3 slots x 34 ef-features stacked on 102 partitions), exp on ACT,
  softmax normalization AFTER aggregation (denominator constant within a
  node's slots), projection + residual on PE/DVE, layernorm batched at the
  end (single Sqrt table load).
- No max-subtraction in softmax (scores are O(10); identical result).
  Padding slots masked via an extra edge-feature column (weight 1, value -75).
- temp/sqrt(d) folded into Wq; temp folded into We; be via a ones column.
"""

import os
import numpy as np
import ml_dtypes

import concourse.bass as bass
import concourse.bacc as bacc
import concourse.tile as tile
from concourse import mybir
from concourse.bass_utils import run_bass_kernel_spmd
from concourse.masks import make_identity

N, E = 50000, 800000
D_NODE, D_EDGE, H = 64, 32, 4
D_H = D_NODE // H
LN_EPS = 1e-5
NCORES = 8
P = 128
NT = 49                # node tiles per core
NPC = NT * P           # padded nodes per core = 6272
KB = 4096              # KV-build rows per DMA batch
KJ = KB // P           # rows per partition per batch
EF_R = 34              # 32 ef features + mask col + ones col (carries be)
EF3 = 3 * EF_R         # 102: three slots stacked on partitions
MASK_VAL = -75.0
F32 = mybir.dt.float32
BF16 = mybir.dt.bfloat16
BF_NP = ml_dtypes.bfloat16


def _col_of_row(r):
    """Inverse of the KV build's column->row permutation.  Build position
    (b, j, p) takes xTc column b*KB + j*P + p to table row b*KB + p*KJ + j
    (per-partition-contiguous DMA writes).  Given the desired row, return
    the column to place the source vector at."""
    b = r // KB
    w = r % KB
    return b * KB + (w % KJ) * P + (w // KJ)


# ---------------------------------------------------------------- host prep --
def _host_prep(node_features, edge_features, edge_index, Wq, bq, Wk, bk, Wv, bv,
               We, be, Wo, bo, ln_gamma, ln_beta, log_temp):
    x = np.ascontiguousarray(np.asarray(node_features, dtype=np.float32))
    ef = np.ascontiguousarray(np.asarray(edge_features, dtype=np.float32))
    src = np.asarray(edge_index[0], dtype=np.int64)
    tgt = np.asarray(edge_index[1], dtype=np.int64)
    temp = np.exp(np.asarray(log_temp, dtype=np.float32))

    deg = np.bincount(tgt, minlength=N)
    order = np.argsort(-deg, kind="stable")
    node_lists = []
    for c in range(NCORES):
        nl = order[c::NCORES]
        nl = np.concatenate([nl, np.full(NPC - len(nl), -1, dtype=np.int64)])
        node_lists.append(nl)

    D_t = np.zeros(NT, dtype=np.int64)
    for c in range(NCORES):
        d = np.where(node_lists[c] >= 0, deg[np.maximum(node_lists[c], 0)], 0)
        D_t = np.maximum(D_t, d.reshape(NT, P).max(axis=1))
    D_t = np.maximum(D_t, 1)
    assert D_t.max() <= 128, f"degree {D_t.max()} exceeds single-bank design"
    SD = int(D_t.sum())
    KC_t = [-(-int(d) // 3) for d in D_t]
    TOT = -(-SD * P // KB) * KB          # padded table rows

    eorder = np.argsort(tgt, kind="stable")
    estart = np.zeros(N + 1, dtype=np.int64)
    np.cumsum(deg, out=estart[1:])

    qscale = (np.repeat(temp, D_H) / np.sqrt(D_H)).astype(np.float32)
    Wq_aug = (np.concatenate([np.asarray(Wq).T, np.asarray(bq)[None, :]], 0)
              * qscale[None, :]).astype(BF_NP)                           # [65,64]
    Wkv_aug = np.concatenate(
        [np.concatenate([np.asarray(Wk).T, np.asarray(Wv).T], 1),
         np.concatenate([np.asarray(bk), np.asarray(bv)])[None, :]], 0
    ).astype(BF_NP)                                                      # [65,128]
    We_augT = np.concatenate(
        [np.asarray(We).T * temp[None, :],
         np.ones((1, H), np.float32),
         (np.asarray(be) * temp)[None, :]], 0
    ).astype(np.float32)                                                 # [34,4]
    We_blk = np.zeros((EF3, 3 * H), dtype=np.float32)
    for j3 in range(3):
        We_blk[j3 * EF_R:(j3 + 1) * EF_R, j3 * H:(j3 + 1) * H] = We_augT
    We_blk = We_blk.astype(BF_NP)
    Wo_aug = np.concatenate(
        [np.asarray(Wo).T, np.asarray(bo)[None, :]], 0).astype(np.float32)  # [65,64]
    gb = np.stack([np.asarray(ln_gamma), np.asarray(ln_beta)]).astype(np.float32)

    x_aug = np.concatenate(
        [x, np.ones((N, 1), np.float32)], 1).astype(BF_NP)               # [N,65]

    SKC = sum(KC_t)
    per_core = []
    for c in range(NCORES):
        nl = node_lists[c]
        efT = np.zeros((EF3, SKC * P), dtype=BF_NP)
        xTc = np.zeros((65, TOT), dtype=BF_NP)
        doff = 0
        koff = 0
        for t in range(NT):
            D = int(D_t[t])
            KC = KC_t[t]
            nlt = nl[t * P:(t + 1) * P]
            degt = np.where(nlt >= 0, deg[np.maximum(nlt, 0)], 0)
            k = np.arange(D)
            valid = k[None, :] < degt[:, None]                    # [P,D]
            pos = estart[np.maximum(nlt, 0)][:, None] + k[None, :]
            eids = eorder[np.minimum(pos, E - 1)]
            eids = np.where(valid, eids, 0)
            gsrc = np.where(valid, src[eids], 0)                  # [P,D]
            # slot (p, k) lives at table row doff*P + p*D + k
            rows = doff * P + (np.arange(P)[:, None] * D + k[None, :])
            cols = _col_of_row(rows)
            xTc[:, cols.ravel()] = x_aug[gsrc.ravel()].T
            blk = np.zeros((P, KC * 3, EF_R), dtype=np.float32)
            blk[:, :, D_EDGE] = MASK_VAL
            blk[:, :D, :D_EDGE] = np.where(valid[:, :, None], ef[eids], 0.0)
            blk[:, :D, D_EDGE] = np.where(valid, 0.0, MASK_VAL)
            blk[:, :, D_EDGE + 1] = 1.0
            # [P, KC, 3, EF_R] -> [3, EF_R, KC, P] -> [102, KC*128]
            efT[:, koff * P:(koff + KC) * P] = (
                blk.reshape(P, KC, 3, EF_R).transpose(2, 3, 1, 0)
                .reshape(EF3, KC * P).astype(BF_NP))
            doff += D
            koff += KC
        xq = np.where(nl[:, None] >= 0, x[np.maximum(nl, 0)], 0.0).astype(np.float32)
        xqT_aug = np.concatenate([xq.T, np.ones((1, NPC), np.float32)],
                                 0).astype(BF_NP)
        xq_g = np.ascontiguousarray(
            xq.reshape(NT, P, D_NODE).transpose(1, 0, 2).reshape(P, NT * D_NODE))
        per_core.append({
            "efT": efT,
            "xTc": xTc,
            "xqT": np.ascontiguousarray(xqT_aug),
            "xq": xq_g,
            "wq": Wq_aug,
            "wkv": np.ascontiguousarray(Wkv_aug),
            "we": np.ascontiguousarray(We_blk),
            "wo": Wo_aug,
            "gb": gb,
        })
    meta = dict(D_seq=[int(d) for d in D_t], TOT=TOT)
    return per_core, node_lists, meta


# ------------------------------------------------------------- bass kernel --
def _build_kernel(meta, debug_mode=None):
    if debug_mode is None:
        debug_mode = os.environ.get("KERNEL_DEBUG_MODE", "")
    D_seq = meta["D_seq"]
    TOT = meta["TOT"]
    SD = sum(D_seq)
    KC_seq = [-(-d // 3) for d in D_seq]
    SKC = sum(KC_seq)
    # eft groups: ~5 DMAs over the run, aligned to tile KC blocks
    NG = 5
    tgt_sz = -(-SKC // NG)
    gsz = []
    acc = 0
    for kc in KC_seq:
        if acc + kc > tgt_sz and acc > 0:
            gsz.append(acc)
            acc = 0
        acc += kc
    gsz.append(acc)
    nc = bacc.Bacc(None, target_bir_lowering=False)

    efT = nc.dram_tensor("efT", [EF3, SKC * P], BF16, kind="ExternalInput")
    xTc = nc.dram_tensor("xTc", [65, TOT], BF16, kind="ExternalInput")
    xqT = nc.dram_tensor("xqT", [65, NPC], BF16, kind="ExternalInput")
    xq = nc.dram_tensor("xq", [P, NT * D_NODE], F32, kind="ExternalInput")
    wq = nc.dram_tensor("wq", [65, D_NODE], BF16, kind="ExternalInput")
    wkv = nc.dram_tensor("wkv", [65, 2 * D_NODE], BF16, kind="ExternalInput")
    we = nc.dram_tensor("we", [EF3, 3 * H], BF16, kind="ExternalInput")
    wo = nc.dram_tensor("wo", [65, D_NODE], F32, kind="ExternalInput")
    gb = nc.dram_tensor("gb", [2, D_NODE], F32, kind="ExternalInput")
    y = nc.dram_tensor("y", [P, NT * D_NODE], F32, kind="ExternalOutput")

    with tile.TileContext(nc) as tc:
        with (
            tc.tile_pool(name="dram", bufs=1, space="DRAM") as dpool,
            tc.tile_pool(name="singles", bufs=1) as singles,
        ):
            kv = dpool.tile([TOT, 2 * D_NODE], BF16)

            wq_sb = singles.tile([65, D_NODE], BF16)
            nc.sync.dma_start(out=wq_sb[:], in_=wq[:])
            wkv_sb = singles.tile([65, 2 * D_NODE], BF16)
            nc.sync.dma_start(out=wkv_sb[:], in_=wkv[:])
            we_sb = singles.tile([EF3, 3 * H], BF16)
            nc.sync.dma_start(out=we_sb[:], in_=we[:])
            wo_sb = singles.tile([65, D_NODE], F32)
            nc.sync.dma_start(out=wo_sb[:], in_=wo[:])
            gamma_sb = singles.tile([P, D_NODE], F32)
            nc.sync.dma_start(
                out=gamma_sb[:],
                in_=bass.AP(tensor=gb[:].tensor, offset=0,
                            ap=[[0, P], [1, D_NODE]]))
            beta_sb = singles.tile([P, D_NODE], F32)
            nc.sync.dma_start(
                out=beta_sb[:],
                in_=bass.AP(tensor=gb[:].tensor, offset=D_NODE,
                            ap=[[0, P], [1, D_NODE]]))
            xqT_sb = singles.tile([65, NPC], BF16)
            nc.sync.dma_start(out=xqT_sb[:], in_=xqT[:])
            xq_sb = singles.tile([P, NT, D_NODE], F32)
            nc.sync.dma_start(out=xq_sb[:], in_=xq[:])
            ident = singles.tile([P, P], F32)
            make_identity(nc, ident[:])
            ones_sb = singles.tile([1, P], F32)
            nc.vector.memset(ones_sb[:], 1.0)
            wob_sb = singles.tile([1, D_NODE], F32)
            nc.sync.dma_start(out=wob_sb[:], in_=wo[64:65, :])
            eps_sb = singles.tile([P, 1], F32)
            nc.vector.memset(eps_sb[:], LN_EPS)
            yout_sb = singles.tile([P, NT, D_NODE], F32)
            mv_sb = singles.tile([P, NT, 2], F32)

            with (
                tc.tile_pool(name="kvb", bufs=3) as kvb,
                tc.tile_pool(name="kvp", bufs=2, space="PSUM") as kvp,
                tc.tile_pool(name="kvg", bufs=2) as kvgp,
                tc.tile_pool(name="eft", bufs=2) as eftp,
                tc.tile_pool(name="mid", bufs=2) as midp,
                tc.tile_pool(name="sml", bufs=3) as smlp,
                tc.tile_pool(name="pq", bufs=1, space="PSUM") as pq,
                tc.tile_pool(name="pb", bufs=2, space="PSUM") as pb,
                tc.tile_pool(name="pt", bufs=1, space="PSUM") as ptp,
                tc.tile_pool(name="py", bufs=2, space="PSUM") as pyp,
            ):
                def build_batch(b):
                    xt_sb = kvb.tile([65, KB], BF16, name="xt_sb")
                    nc.sync.dma_start(
                        out=xt_sb[:], in_=xTc[:, b * KB:(b + 1) * KB])
                    kv_sb = kvb.tile([P, KJ, 2 * D_NODE], BF16, name="kv_sb")
                    for jj in range(KB // 512):
                        pt = kvp.tile([P, 4, 2 * D_NODE], F32, name="pt")
                        for j4 in range(4):
                            j = jj * 4 + j4
                            nc.tensor.matmul(
                                out=pt[:, j4, :],
                                lhsT=xt_sb[:, j * P:(j + 1) * P],
                                rhs=wkv_sb[:], start=True, stop=True)
                        nc.scalar.copy(
                            out=kv_sb[:, jj * 4:(jj + 1) * 4, :], in_=pt[:])
                    # build pos (b, j, p) -> row b*KB + p*KJ + j: per
                    # partition KJ consecutive rows -> contiguous 2KB
                    nc.sync.dma_start(
                        out=bass.AP(
                            tensor=kv[:].tensor,
                            offset=kv[:].offset + b * KB * 2 * D_NODE,
                            ap=[[KJ * 2 * D_NODE, P],
                                [2 * D_NODE, KJ],
                                [1, 2 * D_NODE]]),
                        in_=kv_sb[:])

                built = 0            # batches emitted
                NB = TOT // KB
                doff = 0
                koff = 0
                goff = 0
                gi = 0
                gleft = 0
                eft_sb = None
                for t in range(NT):
                    D = D_seq[t]
                    KC = KC_seq[t]
                    # ensure this tile's kv rows are built
                    need = -(-((doff + D) * P) // KB)
                    while built < min(need, NB):
                        build_batch(built)
                        built += 1
                    if debug_mode == "kv":
                        z = smlp.tile([P, D_NODE], F32, tag="y3", name="z")
                        nc.vector.memset(z[:], 0.0)
                        nc.sync.dma_start(
                            out=y[:, t * D_NODE:(t + 1) * D_NODE], in_=z[:])
                        doff += D
                        koff += KC
                        continue
                    # slot (p, k) at row doff*P + p*D + k: per partition D
                    # consecutive 256B rows -> contiguous
                    kvg = kvgp.tile([P, D, 2 * D_NODE], BF16, tag="kvg",
                                    name="kvg")
                    # issue table reads from the ACT HWDGE queue: keeps the
                    # SP sequencer (build writes + xt loads) off the critical
                    # path of the edge phase
                    nc.scalar.dma_start(
                        out=kvg[:],
                        in_=bass.AP(
                            tensor=kv[:].tensor,
                            offset=kv[:].offset + doff * P * 2 * D_NODE,
                            ap=[[D * 2 * D_NODE, P],
                                [2 * D_NODE, D],
                                [1, 2 * D_NODE]]))
                    if gleft == 0:
                        skc = gsz[gi]
                        eft_sb = eftp.tile([EF3, skc, P], BF16, tag="eft",
                                           name="eft_sb")
                        nc.sync.dma_start(
                            out=eft_sb[:], in_=efT[:, goff * P:(goff + skc) * P])
                        gbase = goff
                        goff += skc
                        gi += 1
                        gleft = skc
                    kbase = koff - gbase

                    # Q' (temp/sqrt(dh) folded) for this tile's 128 nodes
                    qp = pq.tile([P, D_NODE], F32, tag="qp", name="qp")
                    nc.tensor.matmul(out=qp[:], lhsT=xqT_sb[:, t * P:(t + 1) * P],
                                     rhs=wq_sb[:], start=True, stop=True)
                    q_sb = smlp.tile([P, D_NODE], BF16, tag="q", name="q_sb")
                    nc.scalar.copy(out=q_sb[:], in_=qp[:])

                    # per-edge bias: 3 slots per matmul via block-diagonal We
                    biasp = pb.tile([P, 3 * KC, H], F32, tag="biasp", name="biasp")
                    for k in range(KC):
                        nc.tensor.matmul(out=biasp[:, 3 * k:3 * (k + 1), :],
                                         lhsT=eft_sb[:, kbase + k, :],
                                         rhs=we_sb[:], start=True, stop=True)

                    # scores
                    qkp = midp.tile([P, D, H, D_H], BF16, tag="qkp", name="qkp")
                    q_b = bass.AP(tensor=q_sb[:].tensor, offset=q_sb[:].offset,
                                  ap=[q_sb[:].ap[0], [0, D], [1, D_NODE]])
                    nc.vector.tensor_mul(
                        out=qkp[:].rearrange("p d h w -> p d (h w)"),
                        in0=kvg[:, :, 0:D_NODE], in1=q_b)
                    sc = smlp.tile([P, D, H], F32, tag="sc", name="sc")
                    nc.vector.tensor_reduce(
                        out=sc[:], in_=qkp[:], axis=mybir.AxisListType.X,
                        op=mybir.AluOpType.add)
                    sc2 = smlp.tile([P, D, H], F32, tag="sc2", name="sc2")
                    nc.vector.tensor_add(out=sc2[:], in0=sc[:],
                                         in1=biasp[:, 0:D, :])
                    ex = smlp.tile([P, D, H], BF16, tag="ex", name="ex")
                    nc.scalar.activation(out=ex[:], in_=sc2[:],
                                         func=mybir.ActivationFunctionType.Exp)

                    den = smlp.tile([P, H], F32, tag="den", name="den")
                    nc.vector.tensor_reduce(
                        out=den[:], in_=ex[:].rearrange("p d h -> p h d"),
                        axis=mybir.AxisListType.X, op=mybir.AluOpType.add)
                    rden = smlp.tile([P, H], F32, tag="rden", name="rden")
                    nc.vector.tensor_scalar_add(den[:], den[:], 1e-10)
                    nc.vector.reciprocal(out=rden[:], in_=den[:])

                    exv = midp.tile([P, D, H, D_H], BF16, tag="exv", name="exv")
                    nc.vector.tensor_mul(
                        out=exv[:],
                        in0=kvg[:, :, D_NODE:2 * D_NODE].rearrange(
                            "p d (h w) -> p d h w", h=H),
                        in1=ex[:].to_broadcast([P, D, H, D_H]))
                    unn = smlp.tile([P, H, D_H], F32, tag="unn", name="unn")
                    nc.vector.tensor_reduce(
                        out=unn[:], in_=exv[:].rearrange("p d h w -> p h w d"),
                        axis=mybir.AxisListType.X, op=mybir.AluOpType.add)
                    outn = smlp.tile([P, H, D_H], F32, tag="outn", name="outn")
                    nc.vector.tensor_mul(
                        out=outn[:], in0=unn[:],
                        in1=rden[:].to_broadcast([P, H, D_H]))

                    # projection: y1 = outn @ Wo.T + bo + xq
                    tp = ptp.tile([D_NODE, P], F32, tag="tp", name="tp")
                    nc.tensor.transpose(
                        out=tp[:], in_=outn[:].rearrange("p h w -> p (h w)"),
                        identity=ident[:])
                    tps = smlp.tile([D_NODE, P], F32, tag="tps", name="tps")
                    nc.scalar.copy(out=tps[:], in_=tp[:])
                    yp = pyp.tile([P, D_NODE], F32, tag="yp", name="yp")
                    nc.tensor.matmul(out=yp[:], lhsT=tps[:], rhs=wo_sb[0:64, :],
                                     start=True, stop=False)
                    nc.tensor.matmul(out=yp[:], lhsT=ones_sb[:], rhs=wob_sb[:],
                                     start=False, stop=True)
                    nc.vector.tensor_add(out=yout_sb[:, t, :], in0=yp[:],
                                         in1=xq_sb[:, t, :])
                    stats = smlp.tile([P, 6], F32, tag="stats", name="stats")
                    nc.vector.bn_stats(out=stats[:], in_=yout_sb[:, t, :])
                    nc.vector.bn_aggr(out=mv_sb[:, t, :], in_=stats[:])
                    doff += D
                    koff += KC
                    gleft -= KC

                if not debug_mode:
                    # ---- batched layernorm epilogue ----
                    mu = bass.AP(tensor=mv_sb[:].tensor, offset=mv_sb[:].offset,
                                 ap=[mv_sb[:].ap[0], [2, NT]])
                    var = bass.AP(tensor=mv_sb[:].tensor,
                                  offset=mv_sb[:].offset + 1,
                                  ap=[mv_sb[:].ap[0], [2, NT]])
                    sd_sb = singles.tile([P, NT], F32)
                    nc.scalar.activation(out=sd_sb[:], in_=var,
                                         func=mybir.ActivationFunctionType.Sqrt,
                                         bias=eps_sb[:])
                    rsd_sb = singles.tile([P, NT], F32)
                    nc.vector.reciprocal(out=rsd_sb[:], in_=sd_sb[:])
                    mursd_sb = singles.tile([P, NT], F32)
                    nc.vector.tensor_mul(out=mursd_sb[:], in0=mu, in1=rsd_sb[:])

                    def bc_t(a):   # [P, NT] -> [P, NT, 64] (bcast feature)
                        return bass.AP(tensor=a.tensor, offset=a.offset,
                                       ap=list(a.ap) + [[0, D_NODE]])

                    def bc_f(a):   # [P, 64] -> [P, NT, 64] (bcast tile)
                        return bass.AP(tensor=a.tensor, offset=a.offset,
                                       ap=[a.ap[0], [0, NT], a.ap[1]])

                    nc.vector.tensor_mul(out=yout_sb[:], in0=yout_sb[:],
                                         in1=bc_t(rsd_sb[:]))
                    nc.vector.tensor_sub(out=yout_sb[:], in0=yout_sb[:],
                                         in1=bc_t(mursd_sb[:]))
                    nc.vector.tensor_mul(out=yout_sb[:], in0=yout_sb[:],
                                         in1=bc_f(gamma_sb[:]))
                    nc.vector.tensor_add(out=yout_sb[:], in0=yout_sb[:],
                                         in1=bc_f(beta_sb[:]))
                    nc.sync.dma_start(out=y[:], in_=yout_sb[:])

    nc.compile()
    return nc


# ------------------------------------------------------------------ driver --
def kernel(**inputs) -> np.ndarray:
    per_core, node_lists, meta = _host_prep(**inputs)
    nc = _build_kernel(meta)
    res = run_bass_kernel_spmd(nc, per_core, core_ids=list(range(NCORES)))
    y_full = np.zeros((N, D_NODE), dtype=np.float32)
    for c in range(NCORES):
        yc = res.results[c]["y"].reshape(P, NT, D_NODE).transpose(1, 0, 2)
        yc = yc.reshape(NPC, D_NODE)
        nl = node_lists[c]
        real = nl >= 0
        y_full[nl[real]] = yc[real]
    return y_full



# revision 14
# speedup vs baseline: 1.7417x; 1.7417x over previous
"""NodeAttention (GNN scatter-softmax attention) on 8 Trainium2 NeuronCores.

Strategy (v3 — no KV DRAM round trip, supergroup pipeline):
- Host deals nodes to 8 cores round-robin by degree rank; one static NEFF
  serves all cores (SPMD).
- Per core: 49 node-tiles x 128 nodes; tile t has D_t slot-blocks (D_t = max
  degree in tile across cores).  Slot-block (t, k) holds edge k of every node
  in the tile: partition p = node p.  The host replicates per-slot inputs into
  one dense stream xt [98, cols]: rows = x_src(64) | 1 | ef(32) | mask.
- Blocks are processed in supergroups of <=7 within a tile.  Two build
  matmuls per block emit K|bias (68, head-interleaved 16+1) and V (64) into
  separate single-bank PSUM tiles in consume order.  No gather, no KV table
  in DRAM.
- Scores: qk-mul (x bias passthrough via ones in q_aug) routed between
  DVE-direct-from-PSUM and ACT-copy+Pool; DVE reduces 17-wide per head;
  ACT exps one supergroup per pipeline stage.
- Aggregation: per block one PE matmul with identity lhsT and rhs
  [exV(64)|ex(4)] accumulates segment sums AND softmax denominators into
  PSUM across the tile's blocks.  exV routed DVE-direct / ACT-copy+Pool.
- Tail per tile: reciprocal (PSUM direct), normalize, transpose + Wo proj +
  residual via an extra xqT@I matmul (PE), yout copy (ACT); bn stats batched
  at the end; layernorm batched at the end (DVE/Pool split).
- temp/sqrt(d) folded into Wq; temp folded into We; biases via a ones row.
- Software pipeline: stage w emits exp(w-1), builds(w), copies(w), qk(w),
  reduce(w), exv(w-1), agg(w-1) so no engine waits a full round trip.
"""

import numpy as np
import ml_dtypes

import concourse.bass as bass
import concourse.bacc as bacc
import concourse.tile as tile
from concourse import mybir
from concourse.bass_utils import run_bass_kernel_spmd
from concourse.masks import make_identity

N, E = 50000, 800000
D_NODE, D_EDGE, H = 64, 32, 4
D_H = D_NODE // H
LN_EPS = 1e-5
NCORES = 8
P = 128
NT = 49                 # node tiles per core
NPC = NT * P            # padded nodes per core = 6272
XROWS = 98              # x(64) | ones | ef(32) | mask
KBW = 68                # K|bias interleaved: 4 heads x (16 K + 1 bias)
VW = 64
QW = 68                 # q interleaved with ones at bias cols
KB = 8192               # xt columns per DMA batch (64 blocks)
SG = 7                  # max blocks per supergroup (single PSUM bank)
QK_DVE = (1, 0, 1, 0, 0)   # route pattern: 1 = qk-mul DVE-direct
EXV_DVE = (0, 1, 0, 0, 1)  # route pattern: 1 = exv DVE-direct
MASK_VAL = -75.0
F32 = mybir.dt.float32
BF16 = mybir.dt.bfloat16
BF_NP = ml_dtypes.bfloat16


def _sg_sizes(d):
    nsg = -(-d // SG)
    base = d // nsg
    rem = d % nsg
    return [base + (1 if i < rem else 0) for i in range(nsg)]


# ---------------------------------------------------------------- host prep --
def _host_prep(node_features, edge_features, edge_index, Wq, bq, Wk, bk, Wv, bv,
               We, be, Wo, bo, ln_gamma, ln_beta, log_temp):
    x = np.ascontiguousarray(np.asarray(node_features, dtype=np.float32))
    ef = np.ascontiguousarray(np.asarray(edge_features, dtype=np.float32))
    src = np.asarray(edge_index[0], dtype=np.int64)
    tgt = np.asarray(edge_index[1], dtype=np.int64)
    temp = np.exp(np.asarray(log_temp, dtype=np.float32))

    deg = np.bincount(tgt, minlength=N)
    order = np.argsort(-deg, kind="stable")
    node_lists = []
    for c in range(NCORES):
        nl = order[c::NCORES]
        nl = np.concatenate([nl, np.full(NPC - len(nl), -1, dtype=np.int64)])
        node_lists.append(nl)

    D_t = np.zeros(NT, dtype=np.int64)
    for c in range(NCORES):
        d = np.where(node_lists[c] >= 0, deg[np.maximum(node_lists[c], 0)], 0)
        D_t = np.maximum(D_t, d.reshape(NT, P).max(axis=1))
    D_t = np.maximum(D_t, 1)
    assert D_t.max() <= 128, f"degree {D_t.max()} exceeds single-bank design"
    SD = int(D_t.sum())
    TOTC = -(-SD * P // KB) * KB          # xt columns padded to whole batches

    eorder = np.argsort(tgt, kind="stable")
    estart = np.zeros(N + 1, dtype=np.int64)
    np.cumsum(deg, out=estart[1:])

    # Wkvb [98, 132]: per head 16 K cols + 1 bias col, then 64 V cols
    Wkvb = np.zeros((XROWS, KBW + VW), dtype=np.float32)
    Wq_aug = np.zeros((65, QW), dtype=np.float32)
    Wk_, Wv_, Wq_ = np.asarray(Wk), np.asarray(Wv), np.asarray(Wq)
    for h in range(H):
        c0 = 17 * h
        Wkvb[0:64, c0:c0 + 16] = Wk_[16 * h:16 * h + 16, :].T
        Wkvb[64, c0:c0 + 16] = np.asarray(bk)[16 * h:16 * h + 16]
        Wkvb[65:97, c0 + 16] = np.asarray(We)[h, :] * temp[h]
        Wkvb[64, c0 + 16] = np.asarray(be)[h] * temp[h]
        Wkvb[97, c0 + 16] = MASK_VAL * temp[h]
        s = temp[h] / np.sqrt(D_H)
        Wq_aug[0:64, c0:c0 + 16] = Wq_[16 * h:16 * h + 16, :].T * s
        Wq_aug[64, c0:c0 + 16] = np.asarray(bq)[16 * h:16 * h + 16] * s
        Wq_aug[64, c0 + 16] = 1.0
    Wkvb[0:64, KBW:KBW + VW] = Wv_.T
    Wkvb[64, KBW:KBW + VW] = np.asarray(bv)
    Wkvb = Wkvb.astype(BF_NP)
    Wq_aug = Wq_aug.astype(BF_NP)

    Wo_aug = np.concatenate(
        [np.asarray(Wo).T, np.asarray(bo)[None, :]], 0).astype(BF_NP)  # [65,64]
    gb = np.stack([np.asarray(ln_gamma), np.asarray(ln_beta)]).astype(np.float32)

    per_core = []
    for c in range(NCORES):
        nl = node_lists[c]
        xt = np.zeros((XROWS, TOTC), dtype=BF_NP)
        boff = 0
        for t in range(NT):
            D = int(D_t[t])
            nlt = nl[t * P:(t + 1) * P]
            degt = np.where(nlt >= 0, deg[np.maximum(nlt, 0)], 0)
            k = np.arange(D)
            valid = k[None, :] < degt[:, None]                    # [P,D]
            pos = estart[np.maximum(nlt, 0)][:, None] + k[None, :]
            eids = eorder[np.minimum(pos, E - 1)]
            eids = np.where(valid, eids, 0)
            gsrc = np.where(valid, src[eids], 0)                  # [P,D]
            blk = np.zeros((XROWS, D, P), dtype=np.float32)
            blk[0:64] = np.where(
                valid[:, :, None], x[gsrc], 0.0).transpose(2, 1, 0)
            blk[64] = 1.0
            blk[65:97] = np.where(
                valid[:, :, None], ef[eids], 0.0).transpose(2, 1, 0)
            blk[97] = np.where(valid, 0.0, 1.0).T
            xt[:, boff * P:(boff + D) * P] = (
                blk.reshape(XROWS, D * P).astype(BF_NP))
            boff += D
        xq = np.where(nl[:, None] >= 0, x[np.maximum(nl, 0)], 0.0).astype(np.float32)
        xqT_aug = np.concatenate([xq.T, np.ones((1, NPC), np.float32)],
                                 0).astype(BF_NP)
        per_core.append({
            "xt": xt,
            "xqT": np.ascontiguousarray(xqT_aug),
            "wkvb": np.ascontiguousarray(Wkvb),
            "wqa": np.ascontiguousarray(Wq_aug),
            "wo": Wo_aug,
            "gb": gb,
        })
    meta = dict(D_seq=[int(d) for d in D_t], TOTC=TOTC)
    return per_core, node_lists, meta


# ------------------------------------------------------------- bass kernel --
def _build_kernel(meta):
    D_seq = meta["D_seq"]
    TOTC = meta["TOTC"]
    NBATCH = TOTC // KB

    nc = bacc.Bacc(None, target_bir_lowering=False)

    xt = nc.dram_tensor("xt", [XROWS, TOTC], BF16, kind="ExternalInput")
    xqT = nc.dram_tensor("xqT", [65, NPC], BF16, kind="ExternalInput")
    wkvb = nc.dram_tensor("wkvb", [XROWS, KBW + VW], BF16, kind="ExternalInput")
    wqa = nc.dram_tensor("wqa", [65, QW], BF16, kind="ExternalInput")
    wo = nc.dram_tensor("wo", [65, D_NODE], BF16, kind="ExternalInput")
    gb = nc.dram_tensor("gb", [2, D_NODE], F32, kind="ExternalInput")
    y = nc.dram_tensor("y", [P, NT * D_NODE], F32, kind="ExternalOutput")

    # supergroup schedule: (tile, first-block-global, nblocks,
    #                       is_tile_first, is_tile_last)
    sched = []
    boff = 0
    for t in range(NT):
        szs = _sg_sizes(D_seq[t])
        o = 0
        for i, r in enumerate(szs):
            sched.append((t, boff + o, r, i == 0, i == len(szs) - 1))
            o += r
        boff += D_seq[t]

    with tile.TileContext(nc) as tc:
        with tc.tile_pool(name="singles", bufs=1) as singles:
            wkv_sb = singles.tile([XROWS, KBW + VW], BF16)
            nc.scalar.dma_start(out=wkv_sb[:], in_=wkvb[:])
            wq_sb = singles.tile([65, QW], BF16)
            nc.scalar.dma_start(out=wq_sb[:], in_=wqa[:])
            wo_sb = singles.tile([65, D_NODE], BF16)
            nc.scalar.dma_start(out=wo_sb[:], in_=wo[:])
            wob_sb = singles.tile([1, D_NODE], BF16)
            nc.scalar.dma_start(out=wob_sb[:], in_=wo[64:65, :])
            gamma_sb = singles.tile([P, D_NODE], F32)
            nc.scalar.dma_start(
                out=gamma_sb[:],
                in_=bass.AP(tensor=gb[:].tensor, offset=0,
                            ap=[[0, P], [1, D_NODE]]))
            beta_sb = singles.tile([P, D_NODE], F32)
            nc.scalar.dma_start(
                out=beta_sb[:],
                in_=bass.AP(tensor=gb[:].tensor, offset=D_NODE,
                            ap=[[0, P], [1, D_NODE]]))
            xqT_sb = singles.tile([65, NPC], BF16)
            nc.scalar.dma_start(out=xqT_sb[:], in_=xqT[:])
            ident = singles.tile([P, P], BF16)
            make_identity(nc, ident[:])
            ident_f = singles.tile([P, P], F32)
            make_identity(nc, ident_f[:])
            ones_sb = singles.tile([1, P], BF16)
            nc.vector.memset(ones_sb[:], 1.0)
            eps_sb = singles.tile([P, 1], F32)
            nc.vector.memset(eps_sb[:], LN_EPS)
            qaug_sb = singles.tile([P, NT, QW], BF16)
            yout_sb = singles.tile([P, NT, D_NODE], F32)
            mv_sb = singles.tile([P, NT, 2], F32)

            with (
                tc.tile_pool(name="xtp", bufs=4) as xtp,
                tc.tile_pool(name="kbp", bufs=2, space="PSUM") as kbp,
                tc.tile_pool(name="vp", bufs=3, space="PSUM") as vp,
                tc.tile_pool(name="aggp", bufs=2, space="PSUM") as aggp,
                tc.tile_pool(name="misc", bufs=1, space="PSUM") as miscp,
                tc.tile_pool(name="kbs", bufs=2) as kbsp,
                tc.tile_pool(name="vs", bufs=2) as vsp,
                tc.tile_pool(name="qkw", bufs=2) as qkwp,
                tc.tile_pool(name="scw", bufs=3) as scwp,
                tc.tile_pool(name="rhw", bufs=3) as rhwp,
                tc.tile_pool(name="sml", bufs=3) as smlp,
            ):
                xt_bufs = {}

                def xt_buf(b):
                    if b not in xt_bufs:
                        tb = xtp.tile([XROWS, KB], BF16, name="xt_sb")
                        nc.sync.dma_start(out=tb[:], in_=xt[:, b * KB:(b + 1) * KB])
                        xt_bufs[b] = tb
                    return xt_bufs[b]

                agg_t = {}

                def do_qbatch(t0):
                    nt = min(7, NT - t0)
                    qp = kbp.tile([P, SG, KBW], F32, tag="kb", name="qp")
                    for i in range(nt):
                        nc.tensor.matmul(
                            out=qp[:, i, :],
                            lhsT=xqT_sb[:, (t0 + i) * P:(t0 + i + 1) * P],
                            rhs=wq_sb[:], start=True, stop=True)
                    nc.scalar.copy(out=qaug_sb[:, t0:t0 + nt, :],
                                   in_=qp[:, 0:nt, :])

                def do_tail(t):
                    acc = agg_t.pop(t)
                    rden = smlp.tile([P, H], F32, tag="rden", name="rden")
                    nc.vector.reciprocal(out=rden[:], in_=acc[:, 64:68])
                    outn = smlp.tile([P, H, D_H], F32, tag="outn", name="outn")
                    rdb = bass.AP(tensor=rden[:].tensor, offset=rden[:].offset,
                                  ap=list(rden[:].ap) + [[0, D_H]])
                    nc.vector.tensor_mul(
                        out=outn[:],
                        in0=acc[:, 0:64].rearrange("p (h w) -> p h w", h=H),
                        in1=rdb)
                    # one PSUM bank: transpose out [0:64,0:128], proj out [128:192]
                    mt = miscp.tile([P, P + D_NODE], F32, tag="mt", name="mt")
                    nc.tensor.transpose(
                        out=mt[0:D_NODE, 0:P],
                        in_=outn[:].rearrange("p h w -> p (h w)"),
                        identity=ident_f[:])
                    tps = smlp.tile([D_NODE, P], BF16, tag="tps", name="tps")
                    nc.scalar.copy(out=tps[:], in_=mt[0:D_NODE, 0:P])
                    yp = mt[:, P:P + D_NODE]
                    nc.tensor.matmul(out=yp, lhsT=tps[:], rhs=wo_sb[0:64, :],
                                     start=True, stop=False)
                    nc.tensor.matmul(out=yp, lhsT=ones_sb[:],
                                     rhs=wob_sb[:], start=False, stop=False)
                    # residual: += xq via xqT @ I64
                    nc.tensor.matmul(out=yp, lhsT=xqT_sb[0:64, t * P:(t + 1) * P],
                                     rhs=ident[0:64, 0:64], start=False, stop=True)
                    nc.scalar.copy(out=yout_sb[:, t, :], in_=yp)
                    stats = smlp.tile([P, 6], F32, tag="stats", name="stats")
                    nc.vector.bn_stats(out=stats[:], in_=yout_sb[:, t, :])
                    nc.vector.bn_aggr(out=mv_sb[:, t, :], in_=stats[:])

                # pipeline state from previous supergroup
                prev = None
                route_i = 0
                for w, (t, b0, r, tfirst, tlast) in enumerate(sched):
                    if t % 7 == 0 and tfirst:
                        do_qbatch(t)
                    hi = (b0 + r - 1) // 64
                    for bi in range(min(hi + 2, NBATCH - 1) + 1):
                        xt_buf(bi)

                    # stage 1: exp of previous supergroup
                    if prev is not None:
                        (pt, pb0, pr, ptf, ptl, pkv, pvv, psc, prh,
                         pexv_dve) = prev
                        nc.scalar.activation(
                            out=prh[:, 0:pr, 64:68], in_=psc[:, 0:pr, :],
                            func=mybir.ActivationFunctionType.Exp)

                    # stage 2: builds for this supergroup
                    kbt = kbp.tile([P, SG, KBW], F32, tag="kb", name="kbt")
                    vt = vp.tile([P, SG, VW], F32, tag="v", name="vt")
                    for k in range(r):
                        b = b0 + k
                        lt = xt_buf(b // 64)[:, (b % 64) * P:(b % 64 + 1) * P]
                        nc.tensor.matmul(out=kbt[:, k, :], lhsT=lt,
                                         rhs=wkv_sb[:, 0:KBW],
                                         start=True, stop=True)
                        nc.tensor.matmul(out=vt[:, k, :], lhsT=lt,
                                         rhs=wkv_sb[:, KBW:KBW + VW],
                                         start=True, stop=True)

                    qk_dve = QK_DVE[route_i % len(QK_DVE)]
                    exv_dve = EXV_DVE[route_i % len(EXV_DVE)]
                    route_i += 1

                    # stage 3: qk-mul (+ copies if routed via Pool)
                    qab = bass.AP(
                        tensor=qaug_sb[:].tensor,
                        offset=qaug_sb[:].offset + t * QW,
                        ap=[qaug_sb[:].ap[0], [0, r], [1, QW]])
                    qkw = qkwp.tile([P, SG, QW], BF16, tag="qkw", name="qkw")
                    if qk_dve:
                        nc.vector.tensor_mul(out=qkw[:, 0:r, :],
                                             in0=kbt[:, 0:r, :], in1=qab)
                    else:
                        kbs = kbsp.tile([P, SG, KBW], BF16, tag="kbs",
                                        name="kbs")
                        nc.scalar.copy(out=kbs[:, 0:r, :], in_=kbt[:, 0:r, :])
                        nc.gpsimd.tensor_mul(out=qkw[:, 0:r, :],
                                             in0=kbs[:, 0:r, :], in1=qab)
                    if not exv_dve:
                        vs = vsp.tile([P, SG, VW], BF16, tag="vs", name="vs")
                        nc.scalar.copy(out=vs[:, 0:r, :], in_=vt[:, 0:r, :])
                    else:
                        vs = None

                    # stage 4: score reduce
                    scw = scwp.tile([P, SG, H], F32, tag="scw", name="scw")
                    nc.vector.tensor_reduce(
                        out=scw[:, 0:r, :],
                        in_=qkw[:, 0:r, :].rearrange(
                            "p g (h w) -> p g h w", h=H),
                        axis=mybir.AxisListType.X, op=mybir.AluOpType.add)
                    rhw = rhwp.tile([P, SG, QW], BF16, tag="rhw", name="rhw")

                    # stage 5: exv + agg of previous supergroup
                    if prev is not None:
                        (pt, pb0, pr, ptf, ptl, pkv, pvv, psc, prh,
                         pexv_dve) = prev
                        exb = bass.AP(
                            tensor=prh[:].tensor,
                            offset=prh[:].offset + 64,
                            ap=[prh[:].ap[0], [QW, pr], [1, H], [0, D_H]])
                        if pexv_dve:
                            nc.vector.tensor_mul(
                                out=prh[:, 0:pr, 0:64].rearrange(
                                    "p g (h w) -> p g h w", h=H),
                                in0=pvv[:, 0:pr, :].rearrange(
                                    "p g (h w) -> p g h w", h=H),
                                in1=exb)
                        else:
                            nc.gpsimd.tensor_mul(
                                out=prh[:, 0:pr, 0:64].rearrange(
                                    "p g (h w) -> p g h w", h=H),
                                in0=pkv[:, 0:pr, :].rearrange(
                                    "p g (h w) -> p g h w", h=H),
                                in1=exb)
                        if ptf:
                            agg_t[pt] = aggp.tile([P, QW], F32, tag="agg",
                                                  name="agg")
                        for k in range(pr):
                            nc.tensor.matmul(
                                out=agg_t[pt][:], lhsT=ident[:],
                                rhs=prh[:, k, 0:QW],
                                start=(ptf and k == 0),
                                stop=(ptl and k == pr - 1))
                        if ptl:
                            do_tail(pt)

                    prev = (t, b0, r, tfirst, tlast,
                            vs, vt, scw, rhw, exv_dve)

                # flush final supergroup
                (pt, pb0, pr, ptf, ptl, pkv, pvv, psc, prh, pexv_dve) = prev
                nc.scalar.activation(
                    out=prh[:, 0:pr, 64:68], in_=psc[:, 0:pr, :],
                    func=mybir.ActivationFunctionType.Exp)
                exb = bass.AP(
                    tensor=prh[:].tensor,
                    offset=prh[:].offset + 64,
                    ap=[prh[:].ap[0], [QW, pr], [1, H], [0, D_H]])
                if pexv_dve:
                    nc.vector.tensor_mul(
                        out=prh[:, 0:pr, 0:64].rearrange(
                            "p g (h w) -> p g h w", h=H),
                        in0=pvv[:, 0:pr, :].rearrange(
                            "p g (h w) -> p g h w", h=H),
                        in1=exb)
                else:
                    nc.gpsimd.tensor_mul(
                        out=prh[:, 0:pr, 0:64].rearrange(
                            "p g (h w) -> p g h w", h=H),
                        in0=pkv[:, 0:pr, :].rearrange(
                            "p g (h w) -> p g h w", h=H),
                        in1=exb)
                if ptf:
                    agg_t[pt] = aggp.tile([P, QW], F32, tag="agg", name="agg")
                for k in range(pr):
                    nc.tensor.matmul(
                        out=agg_t[pt][:], lhsT=ident[:], rhs=prh[:, k, 0:QW],
                        start=(ptf and k == 0), stop=(ptl and k == pr - 1))
                do_tail(pt)

                # ---- batched layernorm epilogue ----
                mu = bass.AP(tensor=mv_sb[:].tensor, offset=mv_sb[:].offset,
                             ap=[mv_sb[:].ap[0], [2, NT]])
                var = bass.AP(tensor=mv_sb[:].tensor,
                              offset=mv_sb[:].offset + 1,
                              ap=[mv_sb[:].ap[0], [2, NT]])
                sd_sb = singles.tile([P, NT], F32)
                nc.scalar.activation(out=sd_sb[:], in_=var,
                                     func=mybir.ActivationFunctionType.Sqrt,
                                     bias=eps_sb[:])
                rsd_sb = singles.tile([P, NT], F32)
                nc.vector.reciprocal(out=rsd_sb[:], in_=sd_sb[:])
                mursd_sb = singles.tile([P, NT], F32)
                nc.vector.tensor_mul(out=mursd_sb[:], in0=mu, in1=rsd_sb[:])

                def bc_t(a):   # [P, NT] -> [P, NT, 64] (bcast feature)
                    return bass.AP(tensor=a.tensor, offset=a.offset,
                                   ap=list(a.ap) + [[0, D_NODE]])

                def bc_f(a):   # [P, 64] -> [P, NT, 64] (bcast tile)
                    return bass.AP(tensor=a.tensor, offset=a.offset,
                                   ap=[a.ap[0], [0, NT], a.ap[1]])

                nc.gpsimd.tensor_mul(out=yout_sb[:], in0=yout_sb[:],
                                     in1=bc_t(rsd_sb[:]))
                nc.vector.tensor_sub(out=yout_sb[:], in0=yout_sb[:],
                                     in1=bc_t(mursd_sb[:]))
                nc.gpsimd.tensor_mul(out=yout_sb[:], in0=yout_sb[:],
                                     in1=bc_f(gamma_sb[:]))
                nc.vector.tensor_add(out=yout_sb[:], in0=yout_sb[:],
                                     in1=bc_f(beta_sb[:]))
                nc.scalar.dma_start(out=y[:], in_=yout_sb[:])

    nc.compile()
    return nc


# ------------------------------------------------------------------ driver --
def kernel(**inputs) -> np.ndarray:
    per_core, node_lists, meta = _host_prep(**inputs)
    nc = _build_kernel(meta)
    res = run_bass_kernel_spmd(nc, per_core, core_ids=list(range(NCORES)))
    y_full = np.zeros((N, D_NODE), dtype=np.float32)
    for c in range(NCORES):
        yc = res.results[c]["y"].reshape(P, NT, D_NODE).transpose(1, 0, 2)
        yc = yc.reshape(NPC, D_NODE)
        nl = node_lists[c]
        real = nl >= 0
        y_full[nl[real]] = yc[real]
    return y_full


# revision 18
# speedup vs baseline: 1.8202x; 1.0451x over previous
"""NodeAttention (GNN scatter-softmax attention) on 8 Trainium2 NeuronCores.

Strategy (v3 — no KV DRAM round trip, supergroup pipeline):
- Host deals nodes to 8 cores round-robin by degree rank; one static NEFF
  serves all cores (SPMD).
- Per core: 49 node-tiles x 128 nodes; tile t has D_t slot-blocks (D_t = max
  degree in tile across cores).  Slot-block (t, k) holds edge k of every node
  in the tile: partition p = node p.  The host replicates per-slot inputs into
  one dense stream xt [98, cols]: rows = x_src(64) | 1 | ef(32) | mask.
- Blocks are processed in supergroups of <=7 within a tile.  Two build
  matmuls per block emit K|bias (68, head-interleaved 16+1) and V (64) into
  separate single-bank PSUM tiles in consume order.  No gather, no KV table
  in DRAM.
- Scores: qk-mul (x bias passthrough via ones in q_aug) routed between
  DVE-direct-from-PSUM and ACT-copy+Pool; DVE reduces 17-wide per head;
  ACT exps one supergroup per pipeline stage.
- Aggregation: per block one PE matmul with identity lhsT and rhs
  [exV(64)|ex(4)] accumulates segment sums AND softmax denominators into
  PSUM across the tile's blocks.  exV routed DVE-direct / ACT-copy+Pool.
- Tail per tile: reciprocal (PSUM direct), normalize, transpose + Wo proj +
  residual via an extra xqT@I matmul (PE), yout copy (ACT); bn stats batched
  at the end; layernorm batched at the end (DVE/Pool split).
- temp/sqrt(d) folded into Wq; temp folded into We; biases via a ones row.
- Software pipeline: stage w emits exp(w-1), builds(w), copies(w), qk(w),
  reduce(w), exv(w-1), agg(w-1) so no engine waits a full round trip.
"""

import numpy as np
import ml_dtypes

import concourse.bass as bass
import concourse.bacc as bacc
import concourse.tile as tile
from concourse import mybir
from concourse.bass_utils import run_bass_kernel_spmd
from concourse.masks import make_identity

N, E = 50000, 800000
D_NODE, D_EDGE, H = 64, 32, 4
D_H = D_NODE // H
LN_EPS = 1e-5
NCORES = 8
P = 128
NT = 49                 # node tiles per core
NPC = NT * P            # padded nodes per core = 6272
XROWS = 98              # x(64) | ones | ef(32) | mask
KBW = 72                # K|bias interleaved: 4 heads x (16 K + 1 bias + 1 zero)
VW = 64
QW = 72                 # q interleaved with ones at bias cols
RW = 68                 # agg rhs: exV(64) | ex(4)
KB = 8192               # xt columns per DMA batch (64 blocks)
SG = 7                  # max blocks per supergroup (single PSUM bank)
QK_DVE = (1, 0, 1, 0, 0)   # route pattern: 1 = qk-mul DVE-direct
EXV_DVE = (0, 1, 0, 0, 1)  # route pattern: 1 = exv DVE-direct
MASK_VAL = -75.0
F32 = mybir.dt.float32
BF16 = mybir.dt.bfloat16
BF_NP = ml_dtypes.bfloat16


def _sg_sizes(d):
    nsg = -(-d // SG)
    base = d // nsg
    rem = d % nsg
    return [base + (1 if i < rem else 0) for i in range(nsg)]


# ---------------------------------------------------------------- host prep --
def _host_prep(node_features, edge_features, edge_index, Wq, bq, Wk, bk, Wv, bv,
               We, be, Wo, bo, ln_gamma, ln_beta, log_temp):
    x = np.ascontiguousarray(np.asarray(node_features, dtype=np.float32))
    ef = np.ascontiguousarray(np.asarray(edge_features, dtype=np.float32))
    src = np.asarray(edge_index[0], dtype=np.int64)
    tgt = np.asarray(edge_index[1], dtype=np.int64)
    temp = np.exp(np.asarray(log_temp, dtype=np.float32))

    deg = np.bincount(tgt, minlength=N)
    order = np.argsort(-deg, kind="stable")
    node_lists = []
    for c in range(NCORES):
        nl = order[c::NCORES]
        nl = np.concatenate([nl, np.full(NPC - len(nl), -1, dtype=np.int64)])
        node_lists.append(nl)

    D_t = np.zeros(NT, dtype=np.int64)
    for c in range(NCORES):
        d = np.where(node_lists[c] >= 0, deg[np.maximum(node_lists[c], 0)], 0)
        D_t = np.maximum(D_t, d.reshape(NT, P).max(axis=1))
    D_t = np.maximum(D_t, 1)
    assert D_t.max() <= 128, f"degree {D_t.max()} exceeds single-bank design"
    SD = int(D_t.sum())
    TOTC = -(-SD * P // KB) * KB          # xt columns padded to whole batches

    eorder = np.argsort(tgt, kind="stable")
    estart = np.zeros(N + 1, dtype=np.int64)
    np.cumsum(deg, out=estart[1:])

    # Wkvb [98, 132]: per head 16 K cols + 1 bias col, then 64 V cols
    Wkvb = np.zeros((XROWS, KBW + VW), dtype=np.float32)
    Wq_aug = np.zeros((65, QW), dtype=np.float32)
    Wk_, Wv_, Wq_ = np.asarray(Wk), np.asarray(Wv), np.asarray(Wq)
    for h in range(H):
        c0 = 18 * h         # 16 K cols + bias col + zero col per head
        Wkvb[0:64, c0:c0 + 16] = Wk_[16 * h:16 * h + 16, :].T
        Wkvb[64, c0:c0 + 16] = np.asarray(bk)[16 * h:16 * h + 16]
        Wkvb[65:97, c0 + 16] = np.asarray(We)[h, :] * temp[h]
        Wkvb[64, c0 + 16] = np.asarray(be)[h] * temp[h]
        Wkvb[97, c0 + 16] = MASK_VAL * temp[h]
        s = temp[h] / np.sqrt(D_H)
        Wq_aug[0:64, c0:c0 + 16] = Wq_[16 * h:16 * h + 16, :].T * s
        Wq_aug[64, c0:c0 + 16] = np.asarray(bq)[16 * h:16 * h + 16] * s
        Wq_aug[64, c0 + 16] = 1.0
    Wkvb[0:64, KBW:KBW + VW] = Wv_.T
    Wkvb[64, KBW:KBW + VW] = np.asarray(bv)
    Wkvb = Wkvb.astype(BF_NP)
    Wq_aug = Wq_aug.astype(BF_NP)

    Wo_aug = np.concatenate(
        [np.asarray(Wo).T, np.asarray(bo)[None, :]], 0).astype(BF_NP)  # [65,64]
    gb = np.stack([np.asarray(ln_gamma), np.asarray(ln_beta)]).astype(np.float32)

    per_core = []
    for c in range(NCORES):
        nl = node_lists[c]
        xt = np.zeros((XROWS, TOTC), dtype=BF_NP)
        boff = 0
        for t in range(NT):
            D = int(D_t[t])
            nlt = nl[t * P:(t + 1) * P]
            degt = np.where(nlt >= 0, deg[np.maximum(nlt, 0)], 0)
            k = np.arange(D)
            valid = k[None, :] < degt[:, None]                    # [P,D]
            pos = estart[np.maximum(nlt, 0)][:, None] + k[None, :]
            eids = eorder[np.minimum(pos, E - 1)]
            eids = np.where(valid, eids, 0)
            gsrc = np.where(valid, src[eids], 0)                  # [P,D]
            blk = np.zeros((XROWS, D, P), dtype=np.float32)
            blk[0:64] = np.where(
                valid[:, :, None], x[gsrc], 0.0).transpose(2, 1, 0)
            blk[64] = 1.0
            blk[65:97] = np.where(
                valid[:, :, None], ef[eids], 0.0).transpose(2, 1, 0)
            blk[97] = np.where(valid, 0.0, 1.0).T
            xt[:, boff * P:(boff + D) * P] = (
                blk.reshape(XROWS, D * P).astype(BF_NP))
            boff += D
        xq = np.where(nl[:, None] >= 0, x[np.maximum(nl, 0)], 0.0).astype(np.float32)
        xqT_aug = np.concatenate([xq.T, np.ones((1, NPC), np.float32)],
                                 0).astype(BF_NP)
        per_core.append({
            "xt": xt,
            "xqT": np.ascontiguousarray(xqT_aug),
            "wkvb": np.ascontiguousarray(Wkvb),
            "wqa": np.ascontiguousarray(Wq_aug),
            "wo": Wo_aug,
            "gb": gb,
        })
    meta = dict(D_seq=[int(d) for d in D_t], TOTC=TOTC)
    return per_core, node_lists, meta


# ------------------------------------------------------------- bass kernel --
def _build_kernel(meta):
    D_seq = meta["D_seq"]
    TOTC = meta["TOTC"]
    NBATCH = TOTC // KB

    nc = bacc.Bacc(None, target_bir_lowering=False)

    xt = nc.dram_tensor("xt", [XROWS, TOTC], BF16, kind="ExternalInput")
    xqT = nc.dram_tensor("xqT", [65, NPC], BF16, kind="ExternalInput")
    wkvb = nc.dram_tensor("wkvb", [XROWS, KBW + VW], BF16, kind="ExternalInput")
    wqa = nc.dram_tensor("wqa", [65, QW], BF16, kind="ExternalInput")
    wo = nc.dram_tensor("wo", [65, D_NODE], BF16, kind="ExternalInput")
    gb = nc.dram_tensor("gb", [2, D_NODE], F32, kind="ExternalInput")
    y = nc.dram_tensor("y", [P, NT * D_NODE], F32, kind="ExternalOutput")

    # supergroup schedule: (tile, first-block-global, nblocks,
    #                       is_tile_first, is_tile_last)
    sched = []
    boff = 0
    for t in range(NT):
        szs = _sg_sizes(D_seq[t])
        o = 0
        for i, r in enumerate(szs):
            sched.append((t, boff + o, r, i == 0, i == len(szs) - 1))
            o += r
        boff += D_seq[t]

    with tile.TileContext(nc) as tc:
        with tc.tile_pool(name="singles", bufs=1) as singles:
            wkv_sb = singles.tile([XROWS, KBW + VW], BF16)
            nc.scalar.dma_start(out=wkv_sb[:], in_=wkvb[:])
            wq_sb = singles.tile([65, QW], BF16)
            nc.scalar.dma_start(out=wq_sb[:], in_=wqa[:])
            wo_sb = singles.tile([65, D_NODE], BF16)
            nc.scalar.dma_start(out=wo_sb[:], in_=wo[:])
            wob_sb = singles.tile([1, D_NODE], BF16)
            nc.scalar.dma_start(out=wob_sb[:], in_=wo[64:65, :])
            gamma_sb = singles.tile([P, D_NODE], F32)
            nc.scalar.dma_start(
                out=gamma_sb[:],
                in_=bass.AP(tensor=gb[:].tensor, offset=0,
                            ap=[[0, P], [1, D_NODE]]))
            beta_sb = singles.tile([P, D_NODE], F32)
            nc.scalar.dma_start(
                out=beta_sb[:],
                in_=bass.AP(tensor=gb[:].tensor, offset=D_NODE,
                            ap=[[0, P], [1, D_NODE]]))
            xqT_sb = singles.tile([65, NPC], BF16)
            nc.scalar.dma_start(out=xqT_sb[:], in_=xqT[:])
            ident = singles.tile([P, P], BF16)
            make_identity(nc, ident[:])
            ident_f = singles.tile([P, P], F32)
            make_identity(nc, ident_f[:])
            ones_sb = singles.tile([1, P], BF16)
            nc.vector.memset(ones_sb[:], 1.0)
            eps_sb = singles.tile([P, 1], F32)
            nc.vector.memset(eps_sb[:], LN_EPS)
            qaug_sb = singles.tile([P, NT, QW], BF16)
            yout_sb = singles.tile([P, NT, D_NODE], F32)
            mv_sb = singles.tile([P, NT, 2], F32)

            with (
                tc.tile_pool(name="xtp", bufs=4) as xtp,
                tc.tile_pool(name="kbp", bufs=2, space="PSUM") as kbp,
                tc.tile_pool(name="vp", bufs=3, space="PSUM") as vp,
                tc.tile_pool(name="aggp", bufs=2, space="PSUM") as aggp,
                tc.tile_pool(name="misc", bufs=1, space="PSUM") as miscp,
                tc.tile_pool(name="kbs", bufs=2) as kbsp,
                tc.tile_pool(name="vs", bufs=2) as vsp,
                tc.tile_pool(name="qkw", bufs=2) as qkwp,
                tc.tile_pool(name="qkf", bufs=2) as qkfp,
                tc.tile_pool(name="scw", bufs=3) as scwp,
                tc.tile_pool(name="rhw", bufs=3) as rhwp,
                tc.tile_pool(name="sml", bufs=3) as smlp,
            ):
                xt_bufs = {}

                def xt_buf(b):
                    if b not in xt_bufs:
                        tb = xtp.tile([XROWS, KB], BF16, name="xt_sb")
                        nc.sync.dma_start(out=tb[:], in_=xt[:, b * KB:(b + 1) * KB])
                        xt_bufs[b] = tb
                    return xt_bufs[b]

                agg_t = {}

                def do_qbatch(t0):
                    nt = min(7, NT - t0)
                    qp = kbp.tile([P, SG, KBW], F32, tag="kb", name="qp")
                    for i in range(nt):
                        nc.tensor.matmul(
                            out=qp[:, i, :],
                            lhsT=xqT_sb[:, (t0 + i) * P:(t0 + i + 1) * P],
                            rhs=wq_sb[:], start=True, stop=True)
                    nc.scalar.copy(out=qaug_sb[:, t0:t0 + nt, :],
                                   in_=qp[:, 0:nt, :])

                def do_tail(t):
                    acc = agg_t.pop(t)
                    rden = smlp.tile([P, H], F32, tag="rden", name="rden")
                    nc.vector.reciprocal(out=rden[:], in_=acc[:, 64:68])
                    outn = smlp.tile([P, H, D_H], F32, tag="outn", name="outn")
                    rdb = bass.AP(tensor=rden[:].tensor, offset=rden[:].offset,
                                  ap=list(rden[:].ap) + [[0, D_H]])
                    nc.vector.tensor_mul(
                        out=outn[:],
                        in0=acc[:, 0:64].rearrange("p (h w) -> p h w", h=H),
                        in1=rdb)
                    # one PSUM bank: transpose out [0:64,0:128], proj out [128:192]
                    mt = miscp.tile([P, P + D_NODE], F32, tag="mt", name="mt")
                    nc.tensor.transpose(
                        out=mt[0:D_NODE, 0:P],
                        in_=outn[:].rearrange("p h w -> p (h w)"),
                        identity=ident_f[:])
                    tps = smlp.tile([D_NODE, P], BF16, tag="tps", name="tps")
                    nc.scalar.copy(out=tps[:], in_=mt[0:D_NODE, 0:P])
                    yp = mt[:, P:P + D_NODE]
                    nc.tensor.matmul(out=yp, lhsT=tps[:], rhs=wo_sb[0:64, :],
                                     start=True, stop=False)
                    nc.tensor.matmul(out=yp, lhsT=ones_sb[:],
                                     rhs=wob_sb[:], start=False, stop=False)
                    # residual: += xq via xqT @ I64
                    nc.tensor.matmul(out=yp, lhsT=xqT_sb[0:64, t * P:(t + 1) * P],
                                     rhs=ident[0:64, 0:64], start=False, stop=True)
                    nc.scalar.copy(out=yout_sb[:, t, :], in_=yp)
                    stats = smlp.tile([P, 6], F32, tag="stats", name="stats")
                    nc.vector.bn_stats(out=stats[:], in_=yout_sb[:, t, :])
                    nc.vector.bn_aggr(out=mv_sb[:, t, :], in_=stats[:])

                # pipeline state from previous supergroup
                prev = None
                route_i = 0
                for w, (t, b0, r, tfirst, tlast) in enumerate(sched):
                    if t % 7 == 0 and tfirst:
                        do_qbatch(t)
                    hi = (b0 + r - 1) // 64
                    for bi in range(min(hi + 2, NBATCH - 1) + 1):
                        xt_buf(bi)

                    # stage 1: exp of previous supergroup
                    if prev is not None:
                        (pt, pb0, pr, ptf, ptl, pkv, pvv, psc, prh,
                         pexv_dve) = prev
                        nc.scalar.activation(
                            out=prh[:, 0:pr, 64:68], in_=psc[:, 0:pr, :],
                            func=mybir.ActivationFunctionType.Exp)

                    # stage 2: builds for this supergroup
                    kbt = kbp.tile([P, SG, KBW], F32, tag="kb", name="kbt")
                    vt = vp.tile([P, SG, VW], F32, tag="v", name="vt")
                    for k in range(r):
                        b = b0 + k
                        lt = xt_buf(b // 64)[:, (b % 64) * P:(b % 64 + 1) * P]
                        nc.tensor.matmul(out=kbt[:, k, :], lhsT=lt,
                                         rhs=wkv_sb[:, 0:KBW],
                                         start=True, stop=True)
                        nc.tensor.matmul(out=vt[:, k, :], lhsT=lt,
                                         rhs=wkv_sb[:, KBW:KBW + VW],
                                         start=True, stop=True)

                    qk_dve = QK_DVE[route_i % len(QK_DVE)]
                    exv_dve = EXV_DVE[route_i % len(EXV_DVE)]
                    route_i += 1

                    # stage 3: qk-mul (+ copies if routed via Pool)
                    qab = bass.AP(
                        tensor=qaug_sb[:].tensor,
                        offset=qaug_sb[:].offset + t * QW,
                        ap=[qaug_sb[:].ap[0], [0, r], [1, QW]])
                    qkw = qkwp.tile([P, SG, QW], BF16, tag="qkw", name="qkw")
                    if qk_dve:
                        nc.vector.tensor_mul(out=qkw[:, 0:r, :],
                                             in0=kbt[:, 0:r, :], in1=qab)
                    else:
                        kbs = kbsp.tile([P, SG, KBW], BF16, tag="kbs",
                                        name="kbs")
                        nc.scalar.copy(out=kbs[:, 0:r, :], in_=kbt[:, 0:r, :])
                        nc.gpsimd.tensor_mul(out=qkw[:, 0:r, :],
                                             in0=kbs[:, 0:r, :], in1=qab)
                    if not exv_dve:
                        vs = vsp.tile([P, SG, VW], BF16, tag="vs", name="vs")
                        nc.scalar.copy(out=vs[:, 0:r, :], in_=vt[:, 0:r, :])
                    else:
                        vs = None

                    # stage 4: fold halves on Pool, then 9-wide reduce
                    qkf = qkfp.tile([P, SG, H, 9], BF16, tag="qkf", name="qkf")
                    q4 = qkw[:, 0:r, :].rearrange("p g (h w) -> p g h w", h=H)
                    nc.gpsimd.tensor_add(
                        out=qkf[:, 0:r, :, :],
                        in0=q4[:, :, :, 0:9], in1=q4[:, :, :, 9:18])
                    scw = scwp.tile([P, SG, H], F32, tag="scw", name="scw")
                    nc.vector.tensor_reduce(
                        out=scw[:, 0:r, :], in_=qkf[:, 0:r, :, :],
                        axis=mybir.AxisListType.X, op=mybir.AluOpType.add)
                    rhw = rhwp.tile([P, SG, RW], BF16, tag="rhw", name="rhw")

                    # stage 5: exv + agg of previous supergroup
                    if prev is not None:
                        (pt, pb0, pr, ptf, ptl, pkv, pvv, psc, prh,
                         pexv_dve) = prev
                        exb = bass.AP(
                            tensor=prh[:].tensor,
                            offset=prh[:].offset + 64,
                            ap=[prh[:].ap[0], [RW, pr], [1, H], [0, D_H]])
                        if pexv_dve:
                            nc.vector.tensor_mul(
                                out=prh[:, 0:pr, 0:64].rearrange(
                                    "p g (h w) -> p g h w", h=H),
                                in0=pvv[:, 0:pr, :].rearrange(
                                    "p g (h w) -> p g h w", h=H),
                                in1=exb)
                        else:
                            nc.gpsimd.tensor_mul(
                                out=prh[:, 0:pr, 0:64].rearrange(
                                    "p g (h w) -> p g h w", h=H),
                                in0=pkv[:, 0:pr, :].rearrange(
                                    "p g (h w) -> p g h w", h=H),
                                in1=exb)
                        if ptf:
                            agg_t[pt] = aggp.tile([P, RW], F32, tag="agg",
                                                  name="agg")
                        for k in range(pr):
                            nc.tensor.matmul(
                                out=agg_t[pt][:], lhsT=ident[:],
                                rhs=prh[:, k, 0:RW],
                                start=(ptf and k == 0),
                                stop=(ptl and k == pr - 1))
                        if ptl:
                            do_tail(pt)

                    prev = (t, b0, r, tfirst, tlast,
                            vs, vt, scw, rhw, exv_dve)

                # flush final supergroup
                (pt, pb0, pr, ptf, ptl, pkv, pvv, psc, prh, pexv_dve) = prev
                nc.scalar.activation(
                    out=prh[:, 0:pr, 64:68], in_=psc[:, 0:pr, :],
                    func=mybir.ActivationFunctionType.Exp)
                exb = bass.AP(
                    tensor=prh[:].tensor,
                    offset=prh[:].offset + 64,
                    ap=[prh[:].ap[0], [RW, pr], [1, H], [0, D_H]])
                if pexv_dve:
                    nc.vector.tensor_mul(
                        out=prh[:, 0:pr, 0:64].rearrange(
                            "p g (h w) -> p g h w", h=H),
                        in0=pvv[:, 0:pr, :].rearrange(
                            "p g (h w) -> p g h w", h=H),
                        in1=exb)
                else:
                    nc.gpsimd.tensor_mul(
                        out=prh[:, 0:pr, 0:64].rearrange(
                            "p g (h w) -> p g h w", h=H),
                        in0=pkv[:, 0:pr, :].rearrange(
                            "p g (h w) -> p g h w", h=H),
                        in1=exb)
                if ptf:
                    agg_t[pt] = aggp.tile([P, RW], F32, tag="agg", name="agg")
                for k in range(pr):
                    nc.tensor.matmul(
                        out=agg_t[pt][:], lhsT=ident[:], rhs=prh[:, k, 0:RW],
                        start=(ptf and k == 0), stop=(ptl and k == pr - 1))
                do_tail(pt)

                # ---- batched layernorm epilogue ----
                mu = bass.AP(tensor=mv_sb[:].tensor, offset=mv_sb[:].offset,
                             ap=[mv_sb[:].ap[0], [2, NT]])
                var = bass.AP(tensor=mv_sb[:].tensor,
                              offset=mv_sb[:].offset + 1,
                              ap=[mv_sb[:].ap[0], [2, NT]])
                sd_sb = singles.tile([P, NT], F32)
                nc.scalar.activation(out=sd_sb[:], in_=var,
                                     func=mybir.ActivationFunctionType.Sqrt,
                                     bias=eps_sb[:])
                rsd_sb = singles.tile([P, NT], F32)
                nc.vector.reciprocal(out=rsd_sb[:], in_=sd_sb[:])
                mursd_sb = singles.tile([P, NT], F32)
                nc.vector.tensor_mul(out=mursd_sb[:], in0=mu, in1=rsd_sb[:])

                def bc_t(a):   # [P, NT] -> [P, NT, 64] (bcast feature)
                    return bass.AP(tensor=a.tensor, offset=a.offset,
                                   ap=list(a.ap) + [[0, D_NODE]])

                def bc_f(a):   # [P, 64] -> [P, NT, 64] (bcast tile)
                    return bass.AP(tensor=a.tensor, offset=a.offset,
                                   ap=[a.ap[0], [0, NT], a.ap[1]])

                nc.gpsimd.tensor_mul(out=yout_sb[:], in0=yout_sb[:],
                                     in1=bc_t(rsd_sb[:]))
                nc.gpsimd.tensor_sub(out=yout_sb[:], in0=yout_sb[:],
                                     in1=bc_t(mursd_sb[:]))
                nc.gpsimd.tensor_mul(out=yout_sb[:], in0=yout_sb[:],
                                     in1=bc_f(gamma_sb[:]))
                nc.gpsimd.tensor_add(out=yout_sb[:], in0=yout_sb[:],
                                     in1=bc_f(beta_sb[:]))
                nc.sync.dma_start(out=y[:], in_=yout_sb[:])

    nc.compile()
    return nc


# ------------------------------------------------------------------ driver --
def kernel(**inputs) -> np.ndarray:
    per_core, node_lists, meta = _host_prep(**inputs)
    nc = _build_kernel(meta)
    res = run_bass_kernel_spmd(nc, per_core, core_ids=list(range(NCORES)))
    y_full = np.zeros((N, D_NODE), dtype=np.float32)
    for c in range(NCORES):
        yc = res.results[c]["y"].reshape(P, NT, D_NODE).transpose(1, 0, 2)
        yc = yc.reshape(NPC, D_NODE)
        nl = node_lists[c]
        real = nl >= 0
        y_full[nl[real]] = yc[real]
    return y_full
